# revision 17
# baseline (speedup 1.0000x reference)
"""GAT layer (N=16384, d=128) on 8 TRN2 NeuronCores.

Math:
  Wh    = h @ W
  e_src = Wh @ a_src ; e_dst = Wh @ a_dst
  e_ij  = leaky_relu(e_src_i + e_dst_j, 0.01)
  out   = elu(softmax_j(e_ij) @ Wh)

Key identity: exp(leaky_relu(x)) = max(exp(x), exp(0.01 x)), and since
e_ij = s_i + d_j, each unnormalized score tile factors as
  p_ij = max(E_i * F_j, e_i * f_j)
with E=exp(s), e=exp(.01 s) (free-dim vectors) and F=exp(d), f=exp(.01 d)
(per-partition scalars), so the N x N matrix costs only cheap DVE/ACT ops.

Sharding: row-shard the 16384 output rows across 8 cores (2048 each). Every
core sees the full h (rolled so that "its" rows are rows 0..2047) and runs an
identical program: softmax over j is invariant to the j-order, so the roll is
harmless.

On-chip layout (per core):
  score tiles are built transposed, eT[j=partition, i=free], so the
  attention matmul out^T[d, i] = sum_j Wh[j, d] * p[j, i] uses lhsT=Wh tile
  (j on partitions) and rhs=p. The denominator uses lhsT=ones[128,128],
  which lands sum_j p broadcast across all 128 partitions - exactly the
  layout needed to divide out^T by it.
"""

import numpy as np

N, D, P = 16384, 128, 128
N_CORES = 8
ROWS = N // N_CORES  # 2048 output rows per core
NT = N // P  # 128 j-tiles
MY_T = ROWS // P  # 16 chunks of own rows
NEG = 0.01  # leaky_relu slope
DMA_CHUNK = 2048  # hT columns per input DMA

_built = {}


def _build_kernel():
    """Build + compile the Bass module once per process."""
    if "nc" in _built:
        return _built

    import concourse.bass as bass
    import concourse.mybir as mybir
    import concourse.tile as tile
    from concourse import bacc

    f32 = mybir.dt.float32
    bf16 = mybir.dt.bfloat16
    Act = mybir.ActivationFunctionType
    Alu = mybir.AluOpType

    nc = bacc.Bacc("TRN2", target_bir_lowering=False, debug=False)

    hT_d = nc.dram_tensor("hT", [P, N], f32, kind="ExternalInput").ap()
    # [W | W @ a_dst] : 128 x 129, contraction dim (in_dim) on partitions
    wplus_d = nc.dram_tensor("wplus", [P, D + 1], f32, kind="ExternalInput").ap()
    # (W @ a_src) replicated across 128 columns (stationary operand)
    wsrcb_d = nc.dram_tensor("wsrcb", [P, P], f32, kind="ExternalInput").ap()
    ones_d = nc.dram_tensor("ones_bf", [P, P], bf16, kind="ExternalInput").ap()
    outT_d = nc.dram_tensor("outT", [P, ROWS], f32, kind="ExternalOutput").ap()

    with tile.TileContext(nc) as tc:
        with tc.tile_pool(name="singles", bufs=1) as singles:
            # persistent SBUF tensors
            whj = singles.tile([P, N], bf16, tag="whj")  # Wh, j on partitions
            s_raw = singles.tile([P, ROWS], f32, tag="s_raw")  # e_src bcast
            E_b = singles.tile([P, ROWS], bf16, tag="E_b")  # exp(s)
            e_b = singles.tile([P, ROWS], bf16, tag="e_b")  # exp(.01 s)
            edc = singles.tile([P, NT], f32, tag="edc")  # e_dst cols
            F_c = singles.tile([P, NT], f32, tag="F_c")  # exp(e_dst)
            f_c = singles.tile([P, NT], f32, tag="f_c")  # exp(.01 e_dst)
            wplus = singles.tile([P, D + 1], f32, tag="wplus")
            wsrcb = singles.tile([P, P], f32, tag="wsrcb")
            ones_bf = singles.tile([P, P], bf16, tag="ones_bf")

            nc.sync.dma_start(out=wplus, in_=wplus_d)
            nc.sync.dma_start(out=wsrcb, in_=wsrcb_d)
            nc.sync.dma_start(out=ones_bf, in_=ones_d)

            # ---------- Phase 0: Wh (j on partitions), e_dst, e_src ----------
            # Serialized before the main loop on purpose: concurrent DVE/ACT
            # traffic halves DMA throughput (measured), and PSUM cannot hold
            # the 8 accumulator banks plus phase-0 matmul banks anyway.
            with (
                tc.tile_pool(name="hstage", bufs=3) as hstage,
                tc.tile_pool(name="ph0psum", bufs=2, space="PSUM") as ph0psum,
                tc.tile_pool(name="srpsum", bufs=2, space="PSUM") as srpsum,
            ):
                QUAD = 4  # Wh chunks per PSUM tile / per copy
                for blk in range(N // DMA_CHUNK):
                    hts = hstage.tile([P, DMA_CHUNK], f32, tag="hts")
                    nc.sync.dma_start(
                        out=hts, in_=hT_d[:, blk * DMA_CHUNK : (blk + 1) * DMA_CHUNK]
                    )
                    for q in range(DMA_CHUNK // P // QUAD):
                        t0 = blk * (DMA_CHUNK // P) + q * QUAD
                        # 4 matmuls [Wh tile | e_dst col] into one 2-bank PSUM
                        # tile (each 129-col output stays inside a half bank),
                        # then ONE batched strided copy for all 4 Wh tiles.
                        pw = ph0psum.tile([P, QUAD, 256], f32, tag="pw")
                        for k in range(QUAD):
                            t = t0 + k
                            hc = hts[:, (q * QUAD + k) * P : (q * QUAD + k + 1) * P]
                            nc.tensor.matmul(
                                pw[:, k, : D + 1], hc, wplus, start=True, stop=True
                            )
                            if t < MY_T:
                                # e_src for own rows, bcast to all partitions
                                ps = srpsum.tile([P, P], f32, tag="ps")
                                nc.tensor.matmul(ps, wsrcb, hc, start=True, stop=True)
                                nc.vector.tensor_copy(
                                    s_raw[:, t * P : (t + 1) * P], ps
                                )
                        nc.scalar.copy(
                            whj[:, t0 * P : (t0 + QUAD) * P], pw[:, :, :D]
                        )
                        nc.vector.tensor_copy(
                            edc[:, t0 : t0 + QUAD], pw[:, :, D : D + 1]
                        )

            # ---------- Phase 0.5: tiny exp precomputes ----------
            # (kept out of the phase-0 loop: concurrent ACT work measurably
            # throttles the h DMA stream)
            nc.scalar.activation(E_b, s_raw, Act.Exp)
            nc.scalar.activation(e_b, s_raw, Act.Exp, scale=NEG)
            nc.scalar.activation(F_c, edc, Act.Exp)
            nc.scalar.activation(f_c, edc, Act.Exp, scale=NEG)

            # ---------- Main loop over 128 j-tiles ----------
            # Two per-tile strategies, mixed to balance DVE vs ACT:
            #  SA (all-DVE):  t1 = E*F_j ; t2 = e*f_j ; p = max(t1, t2)
            #  SC (all-ACT):  p = exp(lrelu(s + d_j))  [zero DVE work]
            # SC tiles run in blocks of 4 per 11 tiles, Lrelus before Exps, so
            # the ACT spline-table set switches twice per block, not per tile.
            GROUP = 11
            N_SC = 4
            with (
                tc.tile_pool(name="t1pool", bufs=3) as t1pool,
                tc.tile_pool(name="t2pool", bufs=3) as t2pool,
                tc.tile_pool(name="lrpool", bufs=5) as lrpool,
                tc.tile_pool(name="ppool", bufs=6) as ppool,
                tc.tile_pool(name="accpsum", bufs=1, space="PSUM") as accpsum,
            ):
                pnum = accpsum.tile([P, ROWS], f32, tag="pnum")
                pden = accpsum.tile([P, ROWS], f32, tag="pden")

                ptiles = {}

                def emit_mms(t):
                    p = ptiles[t]
                    wt = whj[:, t * P : (t + 1) * P]
                    st, sp = (t == 0), (t == NT - 1)
                    num_mms = [
                        (
                            pnum[:, c * 512 : (c + 1) * 512],
                            wt,
                            p[:, c * 512 : (c + 1) * 512],
                            st,
                            sp,
                        )
                        for c in range(ROWS // 512)
                    ]
                    den_mms = []
                    if t % 2 == 1:
                        # batch den for (t-1, t): one ones-weight-load per pair
                        for tt in (t - 1, t):
                            pp = ptiles.pop(tt)
                            for c in range(ROWS // 512):
                                den_mms.append(
                                    (
                                        pden[:, c * 512 : (c + 1) * 512],
                                        ones_bf,
                                        pp[:, c * 512 : (c + 1) * 512],
                                        tt == 0,
                                        tt == NT - 1,
                                    )
                                )
                    # final pair: den first so the epilogue reciprocal starts
                    # while PE finishes the last num matmuls
                    order = (
                        den_mms + num_mms if t == NT - 1 else num_mms + den_mms
                    )
                    for out_ap, lhs, rhs, st_, sp_ in order:
                        nc.tensor.matmul(out_ap, lhs, rhs, start=st_, stop=sp_)

                for gi, g0 in enumerate(range(0, NT, GROUP)):
                    tiles = list(range(g0, min(g0 + GROUP, NT)))
                    nsc = N_SC if gi % 2 == 0 else N_SC - 1
                    sa = tiles[: max(0, len(tiles) - nsc)]
                    sc = tiles[max(0, len(tiles) - nsc) :]
                    for t in sa:
                        t1 = t1pool.tile([P, ROWS], bf16, tag="t1")
                        nc.vector.tensor_scalar_mul(t1, E_b, F_c[:, t : t + 1])
                        t2 = t2pool.tile([P, ROWS], bf16, tag="t2")
                        nc.vector.tensor_scalar_mul(t2, e_b, f_c[:, t : t + 1])
                        p = ppool.tile([P, ROWS], bf16, tag="p")
                        ptiles[t] = p
                        nc.vector.tensor_max(p, t1, t2)
                    lrs = {}
                    for t in sc:
                        lr = lrpool.tile([P, ROWS], f32, tag="lr")
                        lrs[t] = lr
                        nc.scalar.activation(
                            lr, s_raw, Act.Lrelu, bias=edc[:, t : t + 1], alpha=NEG
                        )
                    for t in sc:
                        p = ppool.tile([P, ROWS], bf16, tag="p")
                        ptiles[t] = p
                        nc.scalar.activation(p, lrs[t], Act.Exp)
                    for t in tiles:
                        emit_mms(t)

                # ---------- Epilogue: divide + ELU (pipelined chunks) --------
                with tc.tile_pool(name="epi", bufs=1) as epi:
                    rden = epi.tile([P, ROWS], f32, tag="rden")
                    htr = epi.tile([P, ROWS], f32, tag="htr")
                    mn = epi.tile([P, ROWS], f32, tag="mn")
                    ex = epi.tile([P, ROWS], f32, tag="ex")
                    rl = epi.tile([P, ROWS], f32, tag="rl")
                    outf = epi.tile([P, ROWS], f32, tag="outf")
                    EC = 512
                    for c in range(ROWS // EC):
                        sl = slice(c * EC, (c + 1) * EC)
                        nc.vector.reciprocal_approx_fast(
                            out=rden[:, sl], in_=pden[:, sl]
                        )
                        nc.vector.tensor_mul(htr[:, sl], pnum[:, sl], rden[:, sl])
                        # elu(x) = relu(x) + exp(min(x,0)) - 1
                        nc.vector.tensor_scalar_min(mn[:, sl], htr[:, sl], 0.0)
                        nc.scalar.activation(ex[:, sl], mn[:, sl], Act.Exp)
                        nc.scalar.activation(rl[:, sl], htr[:, sl], Act.Relu)
                        nc.vector.scalar_tensor_tensor(
                            outf[:, sl],
                            ex[:, sl],
                            -1.0,
                            rl[:, sl],
                            op0=Alu.add,
                            op1=Alu.add,
                        )
                        nc.sync.dma_start(out=outT_d[:, sl], in_=outf[:, sl])

    nc.compile()
    _built["nc"] = nc
    return _built


def kernel(h, W, a_src, a_dst, _trace=False, _trace_kwargs=None):
    import ml_dtypes
    from concourse.bass_utils import run_bass_kernel_spmd

    h = np.asarray(h, dtype=np.float32)
    W = np.asarray(W, dtype=np.float32)
    a_src = np.asarray(a_src, dtype=np.float32)
    a_dst = np.asarray(a_dst, dtype=np.float32)

    built = _build_kernel()
    nc = built["nc"]

    # host-side weight repacking + per-core input layout
    w_src = W @ a_src  # [128]
    w_dst = W @ a_dst  # [128]
    wplus = np.concatenate([W, w_dst[:, None]], axis=1).astype(np.float32)
    wsrcb = np.tile(w_src[:, None], (1, P)).astype(np.float32)
    ones_bf = np.ones((P, P), dtype=ml_dtypes.bfloat16)

    hT = np.ascontiguousarray(h.T)  # [128, N]
    in_maps = []
    for k in range(N_CORES):
        hT_k = np.roll(hT, -k * ROWS, axis=1) if k else hT
        in_maps.append(
            {
                "hT": np.ascontiguousarray(hT_k),
                "wplus": wplus,
                "wsrcb": wsrcb,
                "ones_bf": ones_bf,
            }
        )

    res = run_bass_kernel_spmd(
        nc,
        in_maps,
        core_ids=list(range(N_CORES)),
        trace=_trace,
        **(_trace_kwargs or {}),
    )
    _built["last_result"] = res

    out = np.empty((N, D), dtype=np.float32)
    for k in range(N_CORES):
        out[k * ROWS : (k + 1) * ROWS] = res.results[k]["outT"].T
    return out


# revision 18
# speedup vs baseline: 1.0248x; 1.0248x over previous
"""GAT layer (N=16384, d=128) on 8 TRN2 NeuronCores.

Math:
  Wh    = h @ W
  e_src = Wh @ a_src ; e_dst = Wh @ a_dst
  e_ij  = leaky_relu(e_src_i + e_dst_j, 0.01)
  out   = elu(softmax_j(e_ij) @ Wh)

Key identity: exp(leaky_relu(x)) = max(exp(x), exp(0.01 x)), and since
e_ij = s_i + d_j, each unnormalized score tile factors as
  p_ij = max(E_i * F_j, e_i * f_j)
with E=exp(s), e=exp(.01 s) (free-dim vectors) and F=exp(d), f=exp(.01 d)
(per-partition scalars), so the N x N matrix costs only cheap DVE/ACT ops.

Sharding: row-shard the 16384 output rows across 8 cores (2048 each). Every
core sees the full h (rolled so that "its" rows are rows 0..2047) and runs an
identical program: softmax over j is invariant to the j-order, so the roll is
harmless.

On-chip layout (per core):
  score tiles are built transposed, eT[j=partition, i=free], so the
  attention matmul out^T[d, i] = sum_j Wh[j, d] * p[j, i] uses lhsT=Wh tile
  (j on partitions) and rhs=p. The denominator uses lhsT=ones[128,128],
  which lands sum_j p broadcast across all 128 partitions - exactly the
  layout needed to divide out^T by it.
"""

import numpy as np

N, D, P = 16384, 128, 128
N_CORES = 8
ROWS = N // N_CORES  # 2048 output rows per core
NT = N // P  # 128 j-tiles
MY_T = ROWS // P  # 16 chunks of own rows
NEG = 0.01  # leaky_relu slope
DMA_CHUNK = 2048  # hT columns per input DMA

_built = {}


def _build_kernel():
    """Build + compile the Bass module once per process."""
    if "nc" in _built:
        return _built

    import concourse.bass as bass
    import concourse.mybir as mybir
    import concourse.tile as tile
    from concourse import bacc

    f32 = mybir.dt.float32
    bf16 = mybir.dt.bfloat16
    Act = mybir.ActivationFunctionType
    Alu = mybir.AluOpType

    nc = bacc.Bacc("TRN2", target_bir_lowering=False, debug=False)

    hT_d = nc.dram_tensor("hT", [P, N], f32, kind="ExternalInput").ap()
    # [W | W @ a_dst] : 128 x 129, contraction dim (in_dim) on partitions
    wplus_d = nc.dram_tensor("wplus", [P, D + 1], f32, kind="ExternalInput").ap()
    # (W @ a_src) replicated across 128 columns (stationary operand)
    wsrcb_d = nc.dram_tensor("wsrcb", [P, P], f32, kind="ExternalInput").ap()
    ones_d = nc.dram_tensor("ones_bf", [P, P], bf16, kind="ExternalInput").ap()
    outT_d = nc.dram_tensor("outT", [P, ROWS], f32, kind="ExternalOutput").ap()

    with tile.TileContext(nc) as tc:
        with tc.tile_pool(name="singles", bufs=1) as singles:
            # persistent SBUF tensors
            whj = singles.tile([P, N], bf16, tag="whj")  # Wh, j on partitions
            s_raw = singles.tile([P, ROWS], f32, tag="s_raw")  # e_src bcast
            E_b = singles.tile([P, ROWS], bf16, tag="E_b")  # exp(s)
            e_b = singles.tile([P, ROWS], bf16, tag="e_b")  # exp(.01 s)
            edc = singles.tile([P, NT], f32, tag="edc")  # e_dst cols
            F_c = singles.tile([P, NT], f32, tag="F_c")  # exp(e_dst)
            f_c = singles.tile([P, NT], f32, tag="f_c")  # exp(.01 e_dst)
            wplus = singles.tile([P, D + 1], f32, tag="wplus")
            wsrcb = singles.tile([P, P], f32, tag="wsrcb")
            ones_bf = singles.tile([P, P], bf16, tag="ones_bf")

            nc.sync.dma_start(out=wplus, in_=wplus_d)
            nc.sync.dma_start(out=wsrcb, in_=wsrcb_d)
            nc.sync.dma_start(out=ones_bf, in_=ones_d)

            # ---------- Phase 0: Wh (j on partitions), e_dst, e_src ----------
            # Serialized before the main loop on purpose: concurrent DVE/ACT
            # traffic halves DMA throughput (measured), and PSUM cannot hold
            # the 8 accumulator banks plus phase-0 matmul banks anyway.
            with (
                tc.tile_pool(name="hstage", bufs=3) as hstage,
                tc.tile_pool(name="ph0psum", bufs=3, space="PSUM") as ph0psum,
                tc.tile_pool(name="srpsum", bufs=2, space="PSUM") as srpsum,
            ):
                QUAD = 4  # Wh chunks per PSUM tile / per copy
                for blk in range(N // DMA_CHUNK):
                    hts = hstage.tile([P, DMA_CHUNK], f32, tag="hts")
                    nc.sync.dma_start(
                        out=hts, in_=hT_d[:, blk * DMA_CHUNK : (blk + 1) * DMA_CHUNK]
                    )
                    for q in range(DMA_CHUNK // P // QUAD):
                        t0 = blk * (DMA_CHUNK // P) + q * QUAD
                        # 4 matmuls [Wh tile | e_dst col] into one 2-bank PSUM
                        # tile (each 129-col output stays inside a half bank),
                        # then ONE batched strided copy for all 4 Wh tiles.
                        pw = ph0psum.tile([P, QUAD, 256], f32, tag="pw")
                        for k in range(QUAD):
                            t = t0 + k
                            hc = hts[:, (q * QUAD + k) * P : (q * QUAD + k + 1) * P]
                            nc.tensor.matmul(
                                pw[:, k, : D + 1], hc, wplus, start=True, stop=True
                            )
                            if t < MY_T:
                                # e_src for own rows, bcast to all partitions
                                ps = srpsum.tile([P, P], f32, tag="ps")
                                nc.tensor.matmul(ps, wsrcb, hc, start=True, stop=True)
                                nc.vector.tensor_copy(
                                    s_raw[:, t * P : (t + 1) * P], ps
                                )
                        nc.scalar.copy(
                            whj[:, t0 * P : (t0 + QUAD) * P], pw[:, :, :D]
                        )
                        nc.vector.tensor_copy(
                            edc[:, t0 : t0 + QUAD], pw[:, :, D : D + 1]
                        )

            # ---------- Phase 0.5: tiny exp precomputes ----------
            # (kept out of the phase-0 loop: concurrent ACT work measurably
            # throttles the h DMA stream)
            nc.scalar.activation(E_b, s_raw, Act.Exp)
            nc.scalar.activation(e_b, s_raw, Act.Exp, scale=NEG)
            nc.scalar.activation(F_c, edc, Act.Exp)
            nc.scalar.activation(f_c, edc, Act.Exp, scale=NEG)

            # ---------- Main loop over 128 j-tiles ----------
            # Two per-tile strategies, mixed to balance DVE vs ACT:
            #  SA (all-DVE):  t1 = E*F_j ; t2 = e*f_j ; p = max(t1, t2)
            #  SC (all-ACT):  p = exp(lrelu(s + d_j))  [zero DVE work]
            # SC tiles run in blocks of 4 per 11 tiles, Lrelus before Exps, so
            # the ACT spline-table set switches twice per block, not per tile.
            GROUP = 11
            N_SC = 4
            with (
                tc.tile_pool(name="t1pool", bufs=3) as t1pool,
                tc.tile_pool(name="t2pool", bufs=3) as t2pool,
                tc.tile_pool(name="lrpool", bufs=5) as lrpool,
                tc.tile_pool(name="ppool", bufs=6) as ppool,
                tc.tile_pool(name="accpsum", bufs=1, space="PSUM") as accpsum,
            ):
                pnum = accpsum.tile([P, ROWS], f32, tag="pnum")
                pden = accpsum.tile([P, ROWS], f32, tag="pden")

                ptiles = {}

                def emit_mms(t):
                    p = ptiles[t]
                    wt = whj[:, t * P : (t + 1) * P]
                    st, sp = (t == 0), (t == NT - 1)
                    num_mms = [
                        (
                            pnum[:, c * 512 : (c + 1) * 512],
                            wt,
                            p[:, c * 512 : (c + 1) * 512],
                            st,
                            sp,
                        )
                        for c in range(ROWS // 512)
                    ]
                    den_mms = []
                    if t % 2 == 1:
                        # batch den for (t-1, t): one ones-weight-load per pair
                        for tt in (t - 1, t):
                            pp = ptiles.pop(tt)
                            for c in range(ROWS // 512):
                                den_mms.append(
                                    (
                                        pden[:, c * 512 : (c + 1) * 512],
                                        ones_bf,
                                        pp[:, c * 512 : (c + 1) * 512],
                                        tt == 0,
                                        tt == NT - 1,
                                    )
                                )
                    # final pair: den first so the epilogue reciprocal starts
                    # while PE finishes the last num matmuls
                    order = (
                        den_mms + num_mms if t == NT - 1 else num_mms + den_mms
                    )
                    for out_ap, lhs, rhs, st_, sp_ in order:
                        nc.tensor.matmul(out_ap, lhs, rhs, start=st_, stop=sp_)

                for gi, g0 in enumerate(range(0, NT, GROUP)):
                    tiles = list(range(g0, min(g0 + GROUP, NT)))
                    nsc = N_SC if gi % 2 == 0 else N_SC - 1
                    sa = tiles[: max(0, len(tiles) - nsc)]
                    sc = tiles[max(0, len(tiles) - nsc) :]
                    for t in sa:
                        t1 = t1pool.tile([P, ROWS], bf16, tag="t1")
                        nc.vector.tensor_scalar_mul(t1, E_b, F_c[:, t : t + 1])
                        t2 = t2pool.tile([P, ROWS], bf16, tag="t2")
                        nc.vector.tensor_scalar_mul(t2, e_b, f_c[:, t : t + 1])
                        p = ppool.tile([P, ROWS], bf16, tag="p")
                        ptiles[t] = p
                        nc.vector.tensor_max(p, t1, t2)
                    lrs = {}
                    for t in sc:
                        lr = lrpool.tile([P, ROWS], f32, tag="lr")
                        lrs[t] = lr
                        nc.scalar.activation(
                            lr, s_raw, Act.Lrelu, bias=edc[:, t : t + 1], alpha=NEG
                        )
                    for t in sc:
                        p = ppool.tile([P, ROWS], bf16, tag="p")
                        ptiles[t] = p
                        nc.scalar.activation(p, lrs[t], Act.Exp)
                    for t in tiles:
                        emit_mms(t)

                # ---------- Epilogue: divide + ELU (pipelined chunks) --------
                with tc.tile_pool(name="epi", bufs=1) as epi:
                    rden = epi.tile([P, ROWS], f32, tag="rden")
                    htr = epi.tile([P, ROWS], f32, tag="htr")
                    mn = epi.tile([P, ROWS], f32, tag="mn")
                    ex = epi.tile([P, ROWS], f32, tag="ex")
                    rl = epi.tile([P, ROWS], f32, tag="rl")
                    outf = epi.tile([P, ROWS], f32, tag="outf")
                    EC = 512
                    for c in range(ROWS // EC):
                        sl = slice(c * EC, (c + 1) * EC)
                        nc.vector.reciprocal_approx_fast(
                            out=rden[:, sl], in_=pden[:, sl]
                        )
                        nc.vector.tensor_mul(htr[:, sl], pnum[:, sl], rden[:, sl])
                        # elu(x) = relu(x) + exp(min(x,0)) - 1
                        nc.vector.tensor_scalar_min(mn[:, sl], htr[:, sl], 0.0)
                        nc.scalar.activation(ex[:, sl], mn[:, sl], Act.Exp)
                        nc.scalar.activation(rl[:, sl], htr[:, sl], Act.Relu)
                        nc.vector.scalar_tensor_tensor(
                            outf[:, sl],
                            ex[:, sl],
                            -1.0,
                            rl[:, sl],
                            op0=Alu.add,
                            op1=Alu.add,
                        )
                        nc.sync.dma_start(out=outT_d[:, sl], in_=outf[:, sl])

    nc.compile()
    _built["nc"] = nc
    return _built


def kernel(h, W, a_src, a_dst, _trace=False, _trace_kwargs=None):
    import ml_dtypes
    from concourse.bass_utils import run_bass_kernel_spmd

    h = np.asarray(h, dtype=np.float32)
    W = np.asarray(W, dtype=np.float32)
    a_src = np.asarray(a_src, dtype=np.float32)
    a_dst = np.asarray(a_dst, dtype=np.float32)

    built = _build_kernel()
    nc = built["nc"]

    # host-side weight repacking + per-core input layout
    w_src = W @ a_src  # [128]
    w_dst = W @ a_dst  # [128]
    wplus = np.concatenate([W, w_dst[:, None]], axis=1).astype(np.float32)
    wsrcb = np.tile(w_src[:, None], (1, P)).astype(np.float32)
    ones_bf = np.ones((P, P), dtype=ml_dtypes.bfloat16)

    hT = np.ascontiguousarray(h.T)  # [128, N]
    in_maps = []
    for k in range(N_CORES):
        hT_k = np.roll(hT, -k * ROWS, axis=1) if k else hT
        in_maps.append(
            {
                "hT": np.ascontiguousarray(hT_k),
                "wplus": wplus,
                "wsrcb": wsrcb,
                "ones_bf": ones_bf,
            }
        )

    res = run_bass_kernel_spmd(
        nc,
        in_maps,
        core_ids=list(range(N_CORES)),
        trace=_trace,
        **(_trace_kwargs or {}),
    )
    _built["last_result"] = res

    out = np.empty((N, D), dtype=np.float32)
    for k in range(N_CORES):
        out[k * ROWS : (k + 1) * ROWS] = res.results[k]["outT"].T
    return out
